# revision 2
# baseline (speedup 1.0000x reference)
"""BitLinear v4: fp8(e3m4) prescaled weights, transpose-free, dequant-free.

Host ships signsT pre-transposed and prescaled by scale*64, quantized to
fp8 e3m4 (4 mantissa bits -> ~1.5e-2 rel err, under the 2e-2 gate); the
1/64 is folded into x, which ships pre-transposed in bf16. Device does
per-block accumulating matmuls (lhsT = fp8 sign tile [128k, r<=128o],
rhs = bf16 xT [128k, 32b]) into psum [r, 32b]:
  yT[o,b] = sum_g (s*scale*64)[o,g].T @ (x/64)T[g,b]
Mixed fp8xbf16 matmul is legal on trn2 (both upcast to fp22; only fp32
must match). Halves HBM traffic vs bf16 weights: ~6.1MB/core.
Output is written in SBUF-native layout [128, 11*32]; host reassembles.
"""

import numpy as np

BATCH = 32
IN_F = 4096
OUT_F = 11008
GROUP = 128
N_GROUPS = IN_F // GROUP  # 32
N_CORES = 8
O_SHARD = OUT_F // N_CORES  # 1376
N_BLOCKS = (O_SHARD + 127) // 128  # 11 (10 full + 96 remainder)
CHUNK_O = 256  # o-columns per DMA chunk (2 blocks) -> ~1.05MB per DMA at fp8
N_CHUNKS = (O_SHARD + CHUNK_O - 1) // CHUNK_O  # 6 (last = 96 wide)
IMG_F = N_GROUPS * O_SHARD  # 44032 free bytes per partition (fp8)
W_RESCALE = 64.0  # lift scales into e3m4 normal range [0.25, 15.5)

_nc_cache = []


def _chunk_widths():
    return [min(CHUNK_O, O_SHARD - c * CHUNK_O) for c in range(N_CHUNKS)]


def build_nc():
    import concourse.bacc as bacc
    import concourse.mybir as mybir
    import concourse.tile as tile

    f32 = mybir.dt.float32
    bf16 = mybir.dt.bfloat16
    f8 = mybir.dt.float8e3

    nc = bacc.Bacc(None, target_bir_lowering=False)
    xT_d = nc.dram_tensor("xT", [128, N_GROUPS * BATCH], bf16, kind="ExternalInput")
    sT_d = nc.dram_tensor("signsT", [128, IMG_F], f8, kind="ExternalInput")
    y_d = nc.dram_tensor("y", [128, N_BLOCKS * BATCH], f32, kind="ExternalOutput")

    with tile.TileContext(nc) as tc:
        with tc.tile_pool(name="const", bufs=1) as const, tc.tile_pool(
            name="psum", bufs=1, space="PSUM"
        ) as psum:
            xT = const.tile([128, N_GROUPS, BATCH], bf16, tag="xT")
            y_sb = const.tile([128, N_BLOCKS, BATCH], f32, tag="y_sb")

            nc.sync.dma_start(xT[:], xT_d[:].rearrange("p (g b) -> p g b", g=N_GROUPS))

            s_chunks = []
            off = 0
            for c, w in enumerate(_chunk_widths()):
                sc = const.tile([128, N_GROUPS, w], f8, tag=f"sT{c}")
                nc.sync.dma_start(
                    sc[:],
                    sT_d[:, off : off + N_GROUPS * w].rearrange(
                        "p (g o) -> p g o", g=N_GROUPS
                    ),
                )
                off += N_GROUPS * w
                s_chunks.append(sc)

            # per block: 32 accumulating matmuls into one [r, 32b] psum tile
            for b in range(N_BLOCKS):
                r = min(128, O_SHARD - b * 128)
                sc = s_chunks[b // 2]
                oc = (b % 2) * 128
                ps = psum.tile([128, BATCH], f32, tag="ps", bufs=2)
                for g in range(N_GROUPS):
                    nc.tensor.matmul(
                        ps[:r, :],
                        sc[:, g, oc : oc + r],
                        xT[:, g, :],
                        start=(g == 0),
                        stop=(g == N_GROUPS - 1),
                    )
                nc.vector.tensor_copy(y_sb[:r, b, :], ps[:r, :])

            nc.sync.dma_start(
                y_d[:, 0 : 10 * BATCH].rearrange("p (blk b) -> p blk b", blk=10),
                y_sb[:, 0:10, :],
            )
            nc.sync.dma_start(y_d[0:96, 10 * BATCH :], y_sb[:96, 10, :])
    nc.finalize()
    return nc


def _pack_signs(signs_shard, scales_shard):
    """[O_SHARD, IN_F] +/-1 and [O_SHARD, N_GROUPS] -> prescaled(e3m4) image
    [128, IMG_F], per-chunk contiguous per partition, g-major within chunk."""
    import ml_dtypes

    f8 = ml_dtypes.float8_e3m4
    w_full = signs_shard.astype(np.float32) * np.repeat(
        scales_shard.astype(np.float32) * W_RESCALE, GROUP, axis=1
    )
    sT = w_full.T  # [IN_F, O_SHARD]
    img = np.empty((128, IMG_F), dtype=f8)
    off = 0
    o0 = 0
    for w in _chunk_widths():
        sub = sT[:, o0 : o0 + w].reshape(N_GROUPS, 128, w)
        img[:, off : off + N_GROUPS * w] = (
            sub.transpose(1, 0, 2).reshape(128, N_GROUPS * w).astype(f8)
        )
        off += N_GROUPS * w
        o0 += w
    return img


def _pack_x(x):
    """[BATCH, IN_F] f32 -> xT bf16 [128, N_GROUPS*BATCH] with 1/64 folded."""
    import ml_dtypes

    xt = (np.asarray(x, np.float32) / W_RESCALE).T  # [IN_F, BATCH]
    return np.ascontiguousarray(
        xt.reshape(N_GROUPS, 128, BATCH).transpose(1, 0, 2).reshape(128, -1)
    ).astype(ml_dtypes.bfloat16)


def _shard_inputs(x, scales, signs):
    scales_r = np.asarray(scales, np.float32).reshape(OUT_F, N_GROUPS)
    xT_img = _pack_x(x)
    in_maps = []
    for c in range(N_CORES):
        lo, hi = c * O_SHARD, (c + 1) * O_SHARD
        in_maps.append(
            {
                "xT": xT_img,
                "signsT": _pack_signs(signs[lo:hi], scales_r[lo:hi]),
            }
        )
    return in_maps


def _unshard_out(res):
    cols = []
    for i in range(N_CORES):
        arr = np.asarray(res.results[i]["y"], np.float32)  # [128, 352]
        y_core = arr.reshape(128, N_BLOCKS, BATCH).transpose(1, 0, 2).reshape(
            N_BLOCKS * 128, BATCH
        )[:O_SHARD]
        cols.append(y_core.T)  # [32, 1376]
    return np.ascontiguousarray(np.concatenate(cols, axis=1), dtype=np.float32)


def _run(x, scales, signs, trace=False, tmpdir=None):
    from concourse import bass_utils

    if not _nc_cache:
        _nc_cache.append(build_nc())
    nc = _nc_cache[0]
    in_maps = _shard_inputs(x, scales, signs)
    res = bass_utils.run_bass_kernel_spmd(
        nc, in_maps, list(range(N_CORES)), trace=trace, tmpdir=tmpdir
    )
    return _unshard_out(res), res


def kernel(x, scales, signs):
    out, _ = _run(x, scales, signs)
    return out
